# revision 15
# baseline (speedup 1.0000x reference)
"""BitLinear-1.58 Trainium2 kernel (8-core SPMD).

out = (clip(round(x * s), -128, 127) @ w.T) / s / weight_scale + bias,
s = 127 / clip(rowmax|x|, 1e-5),  w in {0,1} (int32), x [4096, 8192] f32.

Sharding: token dim split 4 ways x out-feature dim split 2 ways -> 8 cores.
Each core: x-block [1024, 8192], weight-block [4096, 8192], out-block [1024, 4096].

Dataflow (all HBM loads are natural/contiguous; transposes via DMA-XBAR):
  x:  quarter loads on the Sync queue one tt ahead of the consuming reduces
      -> rowmax|x| (DVE) -> s = 127*recip(m) -> ACT Copy(x*s + MAGIC)
      -> DVE (-MAGIC, out bf16: RNE integer round) -> XBAR transpose (Sync)
      into the resident xq cache [128k, 64ko, 1024t] bf16 (16 MB).
  w:  stream [128n, 2048k] int32 chunks (Sync) -> int32->bf16 (DVE only; the
      DVE queue carries nothing slower) -> XBAR transpose into fine-grained
      [128k, 8ko, 512n] slab granules (3 live, filled ~2 granules ahead).
  mm: 8 PSUM banks accumulate [128t, 512n] over all 64 ko; each bank drains
      eagerly right after its last accumulation (ACT Copy scale=1/s/ws) and
      stores immediately (ACT queue). The first granule of each n-tile runs
      t-outer so bank revisits match the ~0.8us/bank drain cadence.
  bias: all-zero in this problem spec; if a nonzero bias ever shows up it is
      added on the host (bit-identical op order to the reference, which also
      adds bias after the scaling divides).

Exactness: x_q ints in [-127,127] and w {0,1} are exact in bf16; every partial
sum < 2^24 so fp32 PSUM accumulation is exact. round() = +-1.5*2^23 magic (RNE,
matches jnp.round). clip never binds since |x*s| <= 127 by construction.
s and the output scale use reciprocal-based approximations (<= a few ulp from
the reference's IEEE divides); worst case this flips a knife-edge round() by
one integer step, contributing ~1e-4 relative error vs the 2e-2 budget.
"""
import os as _os
import sys

sys.path.insert(0, "/opt/trn_rl_repo")

from collections import deque
from contextlib import ExitStack

import numpy as np

import concourse.bass as bass
import concourse.tile as tile
from concourse import bacc, mybir
from concourse.bass import ts
from concourse.bass_utils import run_bass_kernel_spmd

TOKENS, IN_F, OUT_F = 4096, 8192, 8192
A_SPLIT, B_SPLIT = 4, 2      # token blocks x outfeature blocks = 8 cores
T_LOC = TOKENS // A_SPLIT    # 1024
N_LOC = OUT_F // B_SPLIT     # 4096
P = 128
KO = IN_F // P               # 64 k-tiles of 128
TT = T_LOC // P              # 8 token tiles
NT = N_LOC // 512            # 8 n-tiles of 512
KQ = 4                       # k quarters (16 ko each): weight load granularity
KO_Q = KO // KQ              # 16
KH = 8                       # ko per slab granule (transpose granularity)
GRAN = KO // KH              # 8 granules per n-tile
NB = 4                       # 128-wide n blocks per 512 n-tile
MAGIC = float(np.float32(1.5 * 2 ** 23))

_NT_DBG = int(_os.environ.get("BITLIN_NT", NT))
_CACHE = {}


def _build():
    if "nc" in _CACHE:
        return _CACHE["nc"]

    nc = bacc.Bacc("TRN2", target_bir_lowering=False, debug=False, num_devices=8)
    f32, bf16, i32 = mybir.dt.float32, mybir.dt.bfloat16, mybir.dt.int32
    A = mybir.AluOpType

    xb = nc.dram_tensor("xb", [T_LOC, IN_F], f32, kind="ExternalInput").ap()
    wb = nc.dram_tensor("wb", [N_LOC, IN_F], i32, kind="ExternalInput").ap()
    ws = nc.dram_tensor("ws", [1], f32, kind="ExternalInput").ap()
    ob = nc.dram_tensor("ob", [T_LOC, N_LOC], f32, kind="ExternalOutput").ap()

    with tile.TileContext(nc) as tc:
        with ExitStack() as ctx:
            small = ctx.enter_context(tc.tile_pool(name="small", bufs=1))
            xqp = ctx.enter_context(tc.tile_pool(name="xq", bufs=1))
            xq = xqp.tile([P, KO, T_LOC], bf16)   # 128 KB/partition, resident

            # weight-scale reciprocal (per-partition [P,1] broadcast)
            ws_sb = small.tile([1, 1], f32)
            nc.sync.dma_start(ws_sb[:], ws[None, :])
            rws = small.tile([1, 1], f32)
            nc.vector.reciprocal(rws[:], ws_sb[:])
            rws_b = small.tile([P, 1], f32)
            nc.gpsimd.partition_broadcast(rws_b[:], rws[:])

            d_all = small.tile([P, TT], f32)      # per-token out scale 1/s/wscale
            m_all = small.tile([P, TT], f32)

            # ---- Phase X: x -> s -> quantize -> XBAR into xq cache ----
            XQRT = 4                     # process x in [128, 2048] quarters
            QW = IN_F // XQRT            # 2048
            with tc.tile_pool(name="phX", bufs=8) as phx, \
                 tc.tile_pool(name="phXq", bufs=3) as phxq:
                xh_tiles = {}

                def pump(t2):
                    if t2 >= TT:
                        return
                    for q2 in range(XQRT):
                        xh = phx.tile([P, QW], f32, tag="xh")
                        nc.sync.dma_start(xh[:], xb[ts(t2, P), ts(q2, QW)])
                        xh_tiles[(t2, q2)] = xh

                def flush(tt, quarters):
                    # rounds (-MAGIC, bf16 out: exact for ints in [-127,127])
                    # split across DVE and ACT, then XBAR into the xq cache
                    for q, xh in enumerate(quarters):
                        xqh = phxq.tile([P, QW], bf16, tag="xqh")
                        if q % 2 == 0:
                            nc.vector.tensor_scalar(xqh[:], xh[:], -MAGIC,
                                                    None, A.add)
                        else:
                            nc.scalar.activation(
                                xqh[:], xh[:],
                                mybir.ActivationFunctionType.Copy, bias=-MAGIC)
                        nc.sync.dma_start_transpose(
                            xq[:, ts(q, KO // XQRT), ts(tt, P)], xqh[:])

                pump(0)
                pump(1)
                staged = None
                for tt in range(TT):
                    quarters = []
                    m4 = small.tile([P, XQRT], f32, tag="m4", name=f"m4_{tt}")
                    for q in range(XQRT):
                        xh = xh_tiles.pop((tt, q))
                        nc.vector.tensor_reduce(
                            m4[:, q : q + 1], xh[:], mybir.AxisListType.X,
                            A.max, apply_absolute_value=True)
                        quarters.append(xh)
                    nc.vector.tensor_reduce(m_all[:, tt : tt + 1], m4[:],
                                            mybir.AxisListType.X, A.max)
                    nc.vector.tensor_scalar_max(m_all[:, tt : tt + 1],
                                                m_all[:, tt : tt + 1], 1e-5)
                    # s = 127 * recip(m); d = m * (1/127) * (1/weight_scale)
                    s_t = small.tile([P, 1], f32, tag="s_t", name=f"s_{tt}")
                    nc.vector.reciprocal(s_t[:], m_all[:, tt : tt + 1])
                    nc.vector.tensor_scalar_mul(s_t[:], s_t[:], 127.0)
                    nc.vector.tensor_scalar(d_all[:, tt : tt + 1],
                                            m_all[:, tt : tt + 1],
                                            float(np.float32(1.0 / 127.0)),
                                            rws_b[:, 0:1], A.mult, A.mult)
                    for q, xh in enumerate(quarters):
                        # ACT: xh = x*s + MAGIC (f32; the +MAGIC snaps the
                        # sum to an integer via RNE at the 2^23 binade)
                        nc.scalar.activation(xh[:], xh[:],
                                             mybir.ActivationFunctionType.Copy,
                                             bias=MAGIC, scale=s_t[:, 0:1])
                    # software pipeline: round+transpose the PREVIOUS tt now,
                    # so the DVE never stalls waiting on this tt's ACT copies
                    if staged is not None:
                        flush(*staged)
                    staged = (tt, quarters)
                    pump(tt + 2)
                flush(*staged)

            # ---- Phase C: stream weight, GEMM, drain ----
            wnp = ctx.enter_context(tc.tile_pool(name="wnat", bufs=3))
            wcp = ctx.enter_context(tc.tile_pool(name="wcvt", bufs=4))
            slp = ctx.enter_context(tc.tile_pool(name="slab", bufs=4))
            pp = ctx.enter_context(tc.tile_pool(name="psum", bufs=8, space="PSUM"))
            op = ctx.enter_context(tc.tile_pool(name="outp", bufs=3))

            # slab granules (8 ko) fill ahead of the MM stream; weight
            # loads+converts happen at 16-ko granularity on even granules,
            # transposes split each converted chunk in half
            fills = [(nt, g) for nt in range(_NT_DBG) for g in range(GRAN)]
            wc_chunks = {}   # (nt, kq) -> list of 4 converted [128,2048] tiles

            def fill_gran(nt, g):
                kq, half = g // 2, g % 2
                if half == 0:
                    chunks = []
                    for nb in range(NB):
                        w_i = wnp.tile([P, P * KO_Q], i32, tag="wi")
                        nc.sync.dma_start(
                            w_i[:], wb[ts(nt * NB + nb, P), ts(kq, P * KO_Q)])
                        w_c = wcp.tile([P, P * KO_Q], bf16, tag="wc")
                        nc.vector.tensor_copy(w_c[:], w_i[:])
                        chunks.append(w_c)
                    wc_chunks[(nt, kq)] = chunks
                slab = slp.tile([P, KH, 512], bf16, tag="slab")
                for nb in range(NB):
                    w_c = wc_chunks[(nt, kq)][nb]
                    nc.sync.dma_start_transpose(
                        slab[:, :, ts(nb, P)], w_c[:, ts(half, KH * P)])
                return slab

            ahead = deque()
            LA = 1
            for i in range(min(LA + 1, len(fills))):
                ahead.append(fill_gran(*fills[i]))
            fi = LA + 1

            for nt in range(_NT_DBG):
                psums = [pp.tile([P, 512], f32, tag="acc", name=f"ps_{nt}_{t}")
                         for t in range(TT)]
                for g in range(GRAN):
                    slab = ahead.popleft()
                    if fi < len(fills):
                        ahead.append(fill_gran(*fills[fi]))
                        fi += 1
                    if g == 0 and nt > 0:
                        # t-outer on the first granule: revisit each PSUM bank
                        # ~1.9us apart, matching the previous n-tile's drain
                        # cadence (ACT runs one 0.8us drain per bank) so the
                        # accumulation restart never stalls on a busy bank
                        for t in range(TT):
                            for kol in range(KH):
                                nc.tensor.matmul(
                                    psums[t][:], xq[:, kol, ts(t, P)],
                                    slab[:, kol, :],
                                    start=(kol == 0), stop=False)
                        continue
                    for kol in range(KH):
                        ko = g * KH + kol
                        last_ko = ko == KO - 1
                        for t in range(TT):
                            nc.tensor.matmul(
                                psums[t][:], xq[:, ko, ts(t, P)], slab[:, kol, :],
                                start=(ko == 0), stop=last_ko)
                            if last_ko:
                                # eager drain: free this PSUM bank right away.
                                # Store on the GpSimd SWDGE queue: HWDGE
                                # stores rotate DMA lanes with slab transposes
                                # (which wait on matmuls) and would block the
                                # ACT queue's drains behind that wait
                                o_sb = op.tile([P, 512], f32, tag="osb")
                                nc.scalar.activation(
                                    o_sb[:], psums[t][:],
                                    mybir.ActivationFunctionType.Copy,
                                    scale=d_all[:, t : t + 1])
                                nc.gpsimd.dma_start(ob[ts(t, P), ts(nt, 512)],
                                                    o_sb[:])

    nc.compile()
    _CACHE["nc"] = nc
    return nc


def kernel(x, weight, weight_scale, bias):
    x = np.ascontiguousarray(np.asarray(x, dtype=np.float32))
    weight = np.ascontiguousarray(np.asarray(weight, dtype=np.int32))
    weight_scale = np.asarray(weight_scale, dtype=np.float32).reshape(1)
    bias = np.ascontiguousarray(np.asarray(bias, dtype=np.float32))

    nc = _build()
    in_maps = []
    for c in range(8):
        i, j = c // B_SPLIT, c % B_SPLIT
        in_maps.append({
            "xb": x[i * T_LOC:(i + 1) * T_LOC],
            "wb": weight[j * N_LOC:(j + 1) * N_LOC],
            "ws": weight_scale,
        })
    res = run_bass_kernel_spmd(nc, in_maps, list(range(8))).results

    out = np.empty((TOKENS, OUT_F), dtype=np.float32)
    for c in range(8):
        i, j = c // B_SPLIT, c % B_SPLIT
        out[i * T_LOC:(i + 1) * T_LOC, j * N_LOC:(j + 1) * N_LOC] = res[c]["ob"]
    if bias.any():
        # reference adds bias after the scaling divides, in f32 — same here
        out = out + bias[None, :].astype(np.float32)
    return out
